# revision 21
# baseline (speedup 1.0000x reference)
"""Trainium2 Bass kernel for the ComplexMixture density-matrix problem.

Math (per batch b), with R = input_real[b] [S, D], I = input_imag[b] [S, D],
w = weight[b] [S]:
    out_r[b] = R^T diag(w) R + I^T diag(w) I      (symmetric)
    out_i[b] = I^T diag(w) R - R^T diag(w) I      (antisymmetric)
Contraction is over S, which maps onto the PE array's partition (K) dim.

Kernel algorithm:
  * 3-multiplication (Karatsuba/Gauss) complex product with w folded in
    via g = sqrt(w) (w >= 0):
        gr = g*R, gin = -g*I   (bf16)
        P1 = gr^T gr,  Q2 = gin^T gin,  P3 = (gr-gin)^T (gr+gin)
        out_r = P1 + Q2;  out_i = P3 - P1 + Q2
  * Hermitian symmetry: only upper-triangular 128-row strips are computed
    (58% of the matmul work); the lower triangle is PE-transposed from
    bf16 copies (bf16 transpose streams 2x faster than fp32, and the
    mirror half tolerates bf16 rounding).
  * Input streaming: the per-core input load takes ~34us at ~365GB/s, so
    batch 0 is loaded in column halves and its matmuls are emitted
    k-outer so the PE consumes each chunk as it lands instead of idling
    (each PE idle also costs a ~3us half-clock p-state ramp).  Batch 1
    loads (full-width chunks) while batch 0 computes; its first strip is
    also k-outer.
  * Output assembly: each 128-row output strip is gathered in a full
    width [128, 768] SBUF row buffer (direct part from the combine,
    mirror part from PE transposes) and written with 3KB-per-line DMAs.
    For batch 1 the mirror part (ready early) and direct part are
    written separately so the kernel doesn't end in one big DMA drain.
  * bf16 operands, fp32 PSUM accumulation (bf16 matmul is 4x fp32 rate).

Sharding: data-parallel over batch B=16 across 8 NeuronCores (2 per core),
no collectives.
"""

import sys

if "/opt/trn_rl_repo" not in sys.path:
    sys.path.insert(0, "/opt/trn_rl_repo")

import numpy as np

# Problem constants (hardcoded per harness contract)
B, S, D = 16, 1024, 768
N_CORES = 8
BPC = B // N_CORES  # batches per core
P = 128
KT = S // P   # 8 k-tiles along S
JT = D // P   # 6 column tiles of 128 along D
HW_COL = 384  # column half width for batch 0's streamed load

# batch 0: column-halved chunks (k0, kn, col0, colw), with 1-k-tile lead
# chunks so the PE starts sooner; batch 1: full width
CHUNKS = {
    0: ([(0, 1, 0, HW_COL), (1, 1, 0, HW_COL)]
        + [(k, 2, 0, HW_COL) for k in range(2, KT, 2)]
        + [(k, 2, HW_COL, HW_COL) for k in range(0, KT, 2)]),
    1: [(k, 2, 0, D) for k in range(0, KT, 2)],
}

# batch-0 strip blocks aligned to the column halves: (m, c0, W)
B0_GROUPS = [
    # (blocks, k_outer, flush_pending, prepB_after, rowdmas_after)
    ([(0, 0, 384), (1, 128, 256)], True, False),
    ([(2, 256, 128)], False, False),
    ([(0, 384, 384), (1, 384, 384)], True, True),
    ([(2, 384, 384), (3, 384, 384)], True, True),
    ([(4, 512, 256)], False, True),
    ([(5, 640, 128)], False, True),
]
B0_ROWDMAS = {2: [0, 1], 3: [2], 4: [3, 4], 5: [5]}  # group idx -> strips


def _strip_blocks(m):
    """Upper-triangular strip m split into PSUM-bank-sized blocks."""
    c0 = m * P
    width = D - c0
    blocks = []
    while width > 0:
        w = min(512, width)
        if width - w == 128 and w == 512:
            w = 384
        blocks.append((m, c0, w))
        c0 += w
        width -= w
    return blocks


_PROGRAM = None


def _build_program():
    import concourse.mybir as mybir
    import concourse.tile as tile
    from concourse import bacc
    from concourse.masks import make_identity

    f32 = mybir.dt.float32
    bf16 = mybir.dt.bfloat16

    nc = bacc.Bacc("TRN2", target_bir_lowering=False, debug=False,
                   num_devices=N_CORES)

    r_dram = nc.dram_tensor("input_real", [BPC, S, D], f32, kind="ExternalInput")
    i_dram = nc.dram_tensor("input_imag", [BPC, S, D], f32, kind="ExternalInput")
    # wg[p, b*KT+k] = sqrt(w[b, k*128+p]); wg[p, 16+b*KT+k] = -sqrt(...)
    # (prepared host-side so the device gets one contiguous DMA)
    wg_dram = nc.dram_tensor("wg", [P, 2 * BPC * KT], f32, kind="ExternalInput")
    or_dram = nc.dram_tensor("out_r", [BPC, D, D], f32, kind="ExternalOutput")
    oi_dram = nc.dram_tensor("out_i", [BPC, D, D], f32, kind="ExternalOutput")

    r_kp = r_dram.ap().rearrange("b (k p) d -> b p k d", p=P)
    i_kp = i_dram.ap().rearrange("b (k p) d -> b p k d", p=P)

    with tile.TileContext(nc) as tc:
        with (
            tc.tile_pool(name="const", bufs=1) as const_pool,
            tc.tile_pool(name="stage", bufs=4) as stage,
            tc.tile_pool(name="big", bufs=2) as big,
            tc.tile_pool(name="rowb", bufs=1) as rowb,
            tc.tile_pool(name="tmp", bufs=3) as tmp,
            tc.tile_pool(name="bfp", bufs=2) as bfp,
            tc.tile_pool(name="psum", bufs=2, space="PSUM") as psum,
            tc.tile_pool(name="psum_t", bufs=2, space="PSUM") as psum_t,
        ):
            wg_sb = const_pool.tile([P, 2 * BPC * KT], f32)
            nc.sync.dma_start(wg_sb[:], wg_dram[:])

            # ---- input DMA issue, all up front -------------------------
            # r chunks on the sync (SP) HWDGE queue, i chunks on gpsimd:
            # descriptor generation for the two streams runs in parallel
            # and batch 0's chunks head both rings.
            stages = {}
            for b in range(BPC):
                for kc, (k0, kn, c0, cw) in enumerate(CHUNKS[b]):
                    ks = slice(k0, k0 + kn)
                    cs = slice(c0, c0 + cw)
                    r32 = stage.tile([P, kn, cw], f32, tag="r32",
                                     padded_shape=[P, 2, D],
                                     name=f"r32_{b}_{kc}")
                    i32 = stage.tile([P, kn, cw], f32, tag="i32",
                                     padded_shape=[P, 2, D],
                                     name=f"i32_{b}_{kc}")
                    nc.sync.dma_start(r32[:], r_kp[b, :, ks, cs])
                    nc.gpsimd.dma_start(i32[:], i_kp[b, :, ks, cs])
                    stages[b, kc] = (r32, i32)

            # identity for the PE transposes (first use is ~20us in)
            ident = const_pool.tile([P, P], bf16)
            make_identity(nc, ident[:])

            ops = {}

            def alloc_bufs(b):
                ops[b] = (
                    big.tile([P, KT, D], bf16, tag="gr", name=f"gr{b}"),
                    big.tile([P, KT, D], bf16, tag="gi", name=f"gi{b}"),
                    big.tile([P, KT, D], bf16, tag="ga", name=f"ga{b}"),
                    big.tile([P, KT, D], bf16, tag="gb", name=f"gb{b}"),
                )

            def emit_prep(b, kc):
                gr, gi, ga, gb = ops[b]
                r32, i32 = stages[b, kc]
                k0, kn, c0, cw = CHUNKS[b][kc]
                cs = slice(c0, c0 + cw)
                for dk in range(kn):
                    k = k0 + dk
                    gcol = wg_sb[:, b * KT + k: b * KT + k + 1]
                    gncol = wg_sb[:, BPC * KT + b * KT + k:
                                  BPC * KT + b * KT + k + 1]
                    nc.vector.tensor_scalar_mul(gr[:, k, cs],
                                                r32[:, dk, :], gcol)
                    nc.scalar.mul(gi[:, k, cs], i32[:, dk, :], gncol)
                    nc.vector.tensor_sub(ga[:, k, cs], gr[:, k, cs],
                                         gi[:, k, cs])
                    nc.vector.tensor_add(gb[:, k, cs], gr[:, k, cs],
                                         gi[:, k, cs])

            pending = []

            def emit_pending():
                for fn in pending:
                    fn()
                pending.clear()

            def mm_group(b, blocks, k_outer, flush=True):
                """Emit the 3 Karatsuba products for a set of blocks.
                Returns [(m, c0, W, (p1, q2, p3))]."""
                gr, gi, ga, gb = ops[b]
                out = []
                for m, c0, W in blocks:
                    out.append((m, c0, W, (
                        psum.tile([P, W], f32, tag="p1", name=f"p1_{b}_{m}_{c0}"),
                        psum.tile([P, W], f32, tag="q2", name=f"q2_{b}_{m}_{c0}"),
                        psum.tile([P, W], f32, tag="p3", name=f"p3_{b}_{m}_{c0}"),
                    )))
                if k_outer:
                    for k in range(KT):
                        st, sp = (k == 0), (k == KT - 1)
                        for pi in range(3):
                            for m, c0, W, pt in out:
                                ms = slice(m * P, (m + 1) * P)
                                cs = slice(c0, c0 + W)
                                lhs, rhs = ((gr, gr), (gi, gi), (ga, gb))[pi]
                                nc.tensor.matmul(pt[pi][:], lhs[:, k, ms],
                                                 rhs[:, k, cs],
                                                 start=st, stop=sp)
                else:
                    for m, c0, W, pt in out:
                        ms = slice(m * P, (m + 1) * P)
                        cs = slice(c0, c0 + W)
                        for pi, (lhs, rhs) in enumerate(
                                ((gr, gr), (gi, gi), (ga, gb))):
                            for k in range(KT):
                                nc.tensor.matmul(pt[pi][:], lhs[:, k, ms],
                                                 rhs[:, k, cs],
                                                 start=(k == 0),
                                                 stop=(k == KT - 1))
                if flush:
                    # previous group's transposes land in the PE queue
                    # behind this group's matmuls (no head-of-line stall)
                    emit_pending()
                return out

            def emit_combine(b, m, c0, W, pt, rowbufs):
                p1, q2, p3 = pt
                cs = slice(c0, c0 + W)
                rbr, rbi = rowbufs[m]
                c1 = tmp.tile([P, W], f32, tag="c1", name=f"c1_{b}_{m}_{c0}")
                ti = tmp.tile([P, W], f32, tag="ti", name=f"ti_{b}_{m}_{c0}")
                nc.scalar.copy(c1[:], p1[:])
                nc.vector.tensor_add(rbr[:, cs], c1[:], q2[:])
                nc.vector.tensor_sub(ti[:], p3[:], c1[:])
                nc.vector.tensor_add(rbi[:, cs], ti[:], q2[:])

                # bf16 mirror sources (cols right of the diagonal block).
                # gpsimd is slow but idle: use it for batch 0; batch 1's
                # mirror chain is on the kernel tail, so it goes on ACT.
                mir0 = max(c0, (m + 1) * P)
                if mir0 >= c0 + W:
                    return
                Wm = c0 + W - mir0
                mirs = slice(mir0, c0 + W)
                orb = bfp.tile([P, Wm], bf16, tag="orb",
                               name=f"orb_{b}_{m}_{c0}")
                oib = bfp.tile([P, Wm], bf16, tag="oib",
                               name=f"oib_{b}_{m}_{c0}")
                if b == 0:
                    nc.gpsimd.tensor_copy(orb[:], rbr[:, mirs])
                    nc.scalar.copy(oib[:], rbi[:, mirs])
                else:
                    nc.scalar.copy(orb[:], rbr[:, mirs])
                    nc.scalar.copy(oib[:], rbi[:, mirs])

                def mk_transposes(m=m, mir0=mir0, Wm=Wm, orb=orb, oib=oib,
                                  rowbufs=rowbufs):
                    mcols = slice(m * P, (m + 1) * P)
                    for j in range(mir0 // P, (mir0 + Wm) // P):
                        off = j * P - mir0
                        tr = psum_t.tile([P, P], bf16, tag="tr")
                        nc.tensor.transpose(tr[:], orb[:, off:off + P],
                                            ident[:])
                        nc.scalar.copy(rowbufs[j][0][:, mcols], tr[:])
                        ti2 = psum_t.tile([P, P], bf16, tag="tr")
                        nc.tensor.transpose(ti2[:], oib[:, off:off + P],
                                            ident[:])
                        nc.scalar.mul(rowbufs[j][1][:, mcols], ti2[:], -1.0)

                pending.append(mk_transposes)

            def row_dma(b, m, rowbufs, cols=None, eng=None):
                ms = slice(m * P, (m + 1) * P)
                cs = slice(0, D) if cols is None else cols
                rbr, rbi = rowbufs[m]
                eng = eng or nc.sync
                eng.dma_start(or_dram[b, ms, cs], rbr[:, cs])
                eng.dma_start(oi_dram[b, ms, cs], rbi[:, cs])

            def alloc_rowbufs(b):
                return [
                    (rowb.tile([P, D], f32, tag=f"rbr{m}", name=f"rbr{b}_{m}"),
                     rowb.tile([P, D], f32, tag=f"rbi{m}", name=f"rbi{b}_{m}"))
                    for m in range(JT)
                ]

            # ================= batch 0: column-halved streaming =========
            alloc_bufs(0)
            for kc in range(5):          # half A prep
                emit_prep(0, kc)
            rowbufs0 = alloc_rowbufs(0)
            b1_prep_iter = iter(range(len(CHUNKS[1])))

            for gi_, (blocks, k_outer, late) in enumerate(B0_GROUPS):
                grp = mm_group(0, blocks, k_outer, flush=(gi_ != 1))
                for m, c0, W, pt in grp:
                    emit_combine(0, m, c0, W, pt, rowbufs0)
                if gi_ == 0:
                    emit_prep(0, 5)      # half B prep, paced with arrival
                    emit_prep(0, 6)
                elif gi_ == 1:
                    emit_prep(0, 7)
                    emit_prep(0, 8)
                elif late:
                    if gi_ == 2:
                        alloc_bufs(1)
                    kc = next(b1_prep_iter, None)
                    if kc is not None:
                        emit_prep(1, kc)
                for m in B0_ROWDMAS.get(gi_, []):
                    row_dma(0, m, rowbufs0)

            for kc in b1_prep_iter:
                emit_prep(1, kc)

            # ================= batch 1: strip-sequential ================
            rowbufs1 = alloc_rowbufs(1)
            for m in range(JT):
                grp = mm_group(1, _strip_blocks(m), k_outer=(m == 0))
                if m >= 1:
                    # mirror part of this row strip is complete (last
                    # contributor was strip m-1, flushed just above):
                    # write it now, from the otherwise-idle gpsimd queue,
                    # so the kernel doesn't end in one monolithic DMA
                    # drain and the sync queue's tail trigger chain stays
                    # short
                    row_dma(1, m, rowbufs1, cols=slice(0, m * P),
                            eng=nc.gpsimd)
                for m_, c0, W, pt in grp:
                    emit_combine(1, m_, c0, W, pt, rowbufs1)
                row_dma(1, m, rowbufs1, cols=slice(m * P, D))
            emit_pending()

    nc.compile()
    return nc


def _get_program():
    global _PROGRAM
    if _PROGRAM is None:
        _PROGRAM = _build_program()
    return _PROGRAM


def kernel(input_real, input_imag, weight, _spmd_kwargs=None):
    input_real = np.ascontiguousarray(input_real, dtype=np.float32)
    input_imag = np.ascontiguousarray(input_imag, dtype=np.float32)
    weight = np.ascontiguousarray(weight, dtype=np.float32)

    from concourse.bass_utils import run_bass_kernel_spmd

    nc = _get_program()
    # wg[p, b*KT+k] = sqrt(w[b, k*128+p]), second half negated (host-side
    # prep so the device gets one contiguous DMA and no sqrt chain)
    g = np.sqrt(weight).reshape(B, KT, P).transpose(2, 0, 1).reshape(P, B, KT)
    in_maps = []
    for c in range(N_CORES):
        lo, hi = c * BPC, (c + 1) * BPC
        gc = g[:, lo:hi, :].reshape(P, BPC * KT)
        in_maps.append({
            "input_real": input_real[lo:hi],
            "input_imag": input_imag[lo:hi],
            "wg": np.ascontiguousarray(
                np.concatenate([gc, -gc], axis=1), dtype=np.float32),
        })
    res = run_bass_kernel_spmd(nc, in_maps, list(range(N_CORES)),
                               **(_spmd_kwargs or {}))
    out_r = np.concatenate([res.results[c]["out_r"] for c in range(N_CORES)], 0)
    out_i = np.concatenate([res.results[c]["out_i"] for c in range(N_CORES)], 0)
    kernel.last_results = res
    return (out_r, out_i)
